# revision 3
# baseline (speedup 1.0000x reference)
"""MinGRU Trainium2 kernel.

Math (linear-space reformulation of the reference's log-space scan; all
quantities are positive so this is numerically safe):
    k = x @ W_z.T ; u = x @ W_h.T
    c_t = sigmoid(-k_t)            # decay coeff (1 - z_t)
    v_t = sigmoid(k_t) * g(u_t)    # input term, g(u) = relu(u) + sigmoid(min(u,0))
    h_t = c_t * h_{t-1} + v_t      # hardware tensor_tensor_scan (mult, add)
    h_0 = g(h0)

Sharding: 8 cores = 4 batches x 2 halves of d_model output channels.
Each core: x[b].T [1024,4096] fp32, weight slices W.T[:, half] [1024,512],
out h.T [512,4096].  Matmuls run as float32r (FP22, full PE rate), scan on
the Vector engine along the time (free) axis, 128 channels per partition.
"""

import numpy as np

B, T, D = 4, 4096, 1024
EC = 512            # output channels per core
ET = EC // 128      # 4 e-tiles per core
KT = D // 128       # 8 k-tiles
TC = 512            # time chunk (columns per matmul / PSUM bank)
NCHUNK = T // TC    # 8

_CACHED = {}
LAST_RESULT = None


def _build_nc():
    import concourse.bass as bass
    import concourse.bacc as bacc
    import concourse.mybir as mybir
    import concourse.tile as tile

    f32 = mybir.dt.float32
    f32r = mybir.dt.float32r
    AF = mybir.ActivationFunctionType
    OP = mybir.AluOpType

    nc = bacc.Bacc(None, target_bir_lowering=False)

    xT = nc.dram_tensor("xT", [D, T], f32, kind="ExternalInput")
    wz = nc.dram_tensor("wzT", [D, EC], f32, kind="ExternalInput")
    wh = nc.dram_tensor("whT", [D, EC], f32, kind="ExternalInput")
    h0g = nc.dram_tensor("h0g", [128, ET], f32, kind="ExternalInput")
    hT = nc.dram_tensor("hT", [EC, T], f32, kind="ExternalOutput")

    xT_ap = xT[:].rearrange("(kt p) t -> p kt t", p=128)
    wz_ap = wz[:].rearrange("(kt p) e -> p kt e", p=128)
    wh_ap = wh[:].rearrange("(kt p) e -> p kt e", p=128)

    with tile.TileContext(nc) as tc:
        with (
            tc.tile_pool(name="wpool", bufs=1) as wpool,
            tc.tile_pool(name="xpool", bufs=2) as xpool,
            tc.tile_pool(name="work", bufs=2) as work,
            tc.tile_pool(name="hpool", bufs=2) as hpool,
            tc.tile_pool(name="psum", bufs=1, space=bass.MemorySpace.PSUM) as psum,
        ):
            wz_sb = wpool.tile([128, KT, EC], f32r, tag="wz")
            wh_sb = wpool.tile([128, KT, EC], f32r, tag="wh")
            h0_sb = wpool.tile([128, ET], f32, tag="h0")
            nc.sync.dma_start(out=wz_sb[:], in_=wz_ap.bitcast(f32r))
            nc.sync.dma_start(out=wh_sb[:], in_=wh_ap.bitcast(f32r))
            nc.sync.dma_start(out=h0_sb[:], in_=h0g[:])

            h_prev = [None] * ET
            for ci in range(NCHUNK):
                tsl = slice(ci * TC, (ci + 1) * TC)
                x_sb = xpool.tile([128, KT, TC], f32r, tag="x")
                nc.sync.dma_start(out=x_sb[:], in_=xT_ap[:, :, tsl].bitcast(f32r))

                for e in range(ET):
                    esl = slice(e * 128, (e + 1) * 128)
                    pk = psum.tile([128, TC], f32, tag=f"pk{e}")
                    pu = psum.tile([128, TC], f32, tag=f"pu{e}")
                    for kt in range(KT):
                        nc.tensor.matmul(
                            pk[:],
                            wz_sb[:, kt, esl],
                            x_sb[:, kt, :],
                            start=(kt == 0),
                            stop=(kt == KT - 1),
                        )
                    for kt in range(KT):
                        nc.tensor.matmul(
                            pu[:],
                            wh_sb[:, kt, esl],
                            x_sb[:, kt, :],
                            start=(kt == 0),
                            stop=(kt == KT - 1),
                        )

                    c = work.tile([128, TC], f32, tag=f"c{e}")
                    z = work.tile([128, TC], f32, tag=f"z{e}")
                    m = work.tile([128, TC], f32, tag=f"m{e}")
                    s = work.tile([128, TC], f32, tag=f"s{e}")
                    g = work.tile([128, TC], f32, tag=f"g{e}")
                    v = work.tile([128, TC], f32, tag=f"v{e}")
                    h = hpool.tile([128, TC], f32, tag=f"h{e}")

                    # c = sigmoid(-k); z = sigmoid(k)
                    nc.scalar.activation(c[:], pk[:], AF.Sigmoid, scale=-1.0)
                    nc.scalar.activation(z[:], pk[:], AF.Sigmoid, scale=1.0)
                    # g = relu(u) + sigmoid(min(u, 0))
                    nc.vector.tensor_scalar_min(m[:], pu[:], 0.0)
                    nc.scalar.activation(s[:], m[:], AF.Sigmoid, scale=1.0)
                    nc.vector.scalar_tensor_tensor(
                        g[:], pu[:], 0.0, s[:], op0=OP.max, op1=OP.add
                    )
                    # v = z * g
                    nc.vector.tensor_mul(v[:], z[:], g[:])
                    # h_t = c_t * h_{t-1} + v_t
                    init = h0_sb[:, e : e + 1] if ci == 0 else h_prev[e][:, TC - 1 : TC]
                    nc.vector.tensor_tensor_scan(
                        h[:], c[:], v[:], init, op0=OP.mult, op1=OP.add
                    )
                    h_prev[e] = h
                    nc.sync.dma_start(out=hT[esl, tsl], in_=h[:])

    nc.compile()
    return nc


def _get_nc():
    if "nc" not in _CACHED:
        _CACHED["nc"] = _build_nc()
    return _CACHED["nc"]


def kernel(x, h0, W_h, W_z, _trace=False):
    global LAST_RESULT
    from concourse import bass_utils

    x = np.asarray(x, np.float32)
    h0 = np.asarray(h0, np.float32)
    W_h = np.asarray(W_h, np.float32)
    W_z = np.asarray(W_z, np.float32)

    # host-side prep: transposes + initial state g(h0)
    gh0 = np.where(h0 >= 0, h0 + np.float32(0.5),
                   1.0 / (1.0 + np.exp(-h0))).astype(np.float32)  # [B,1,D]
    WzT = np.ascontiguousarray(W_z.T)  # [D, D] (in-dim, out-dim)
    WhT = np.ascontiguousarray(W_h.T)

    in_maps = []
    for b in range(B):
        xTb = np.ascontiguousarray(x[b].T)  # [D, T]
        for eh in range(2):
            esl = slice(eh * EC, (eh + 1) * EC)
            h0c = np.ascontiguousarray(
                gh0[b, 0, esl].reshape(ET, 128).T)  # [128, ET]
            in_maps.append({
                "xT": xTb,
                "wzT": np.ascontiguousarray(WzT[:, esl]),
                "whT": np.ascontiguousarray(WhT[:, esl]),
                "h0g": h0c,
            })

    nc = _get_nc()
    res = bass_utils.run_bass_kernel_spmd(
        nc, in_maps, core_ids=list(range(8)), trace=_trace,
    )
    LAST_RESULT = res

    out = np.empty((B, T, D), np.float32)
    for b in range(B):
        for eh in range(2):
            core = b * 2 + eh
            out[b, :, eh * EC:(eh + 1) * EC] = res.results[core]["hT"].T
    return out
